# revision 38
# baseline (speedup 1.0000x reference)
"""Multi-head attention (B=2, S=2048, D=1024, H=16, dh=64) on 8 Trainium2 cores.

Sharding: head-tensor-parallel x batch. Core c owns batch b=c//4 and heads
4*(c%4)..4*(c%4)+3 (256 of the 1024 ctx dims). Each core computes its heads'
Q/K/V projections, attention, and a partial output projection against its
256 rows of Wo (+ bo/4 so the 4 partials per batch sum to one bo). The host
unshard step sums the 4 partial outputs per batch (fp16 partials, fp32 sum)
— the tensor-parallel all-reduce of the sharding hint, done at gather time.

Per-core kernel (fp16 matmul operands, fp32 PSUM accumulation):
  qT/kT [256e, 2048t] = W.T @ x.T computed directly in transposed form so
  scores^T [kt, qt] = (kT slice).T @ (qT slice) needs no on-device transpose.
  Head pairs are row-packed (heads at array rows 0-63 / 64-127) so the two
  score matmuls of a pair overlap in the PE array. exp runs on ScalarE with
  the 1/sqrt(dh) scale folded in. A@V uses a stationary operand [V | 1]
  (per-head ones column, pre-baked into the SBUF v tiles via the eviction
  addend) so the softmax denominator falls out of the same matmul.

  Softmax normalization runs entirely off the ACT critical path: ctx pairs
  are evicted PSUM->SBUF with fast fp32 copies, 1/rowsum comes from the DVE
  custom-op reciprocal_approx_fast, the row is broadcast across partitions
  by GpSimdE (partition_broadcast), and a deferred DVE multiply produces the
  normalized ctx. ScalarE runs nothing but the exp stream.

  The emission is a single software-pipelined stream over the 128 (slice, m)
  steps with a depth-1 skew: step s emits fillers, scores(s)+exp(s), then
  ctx(s-1) — so ctx matmuls never wait on the exp semaphore at the head of
  the PE queue. Projection / out-projection groups are placed as fillers
  with explicit deadlines; tiny bias DMAs are issued before the bulk x/W
  transfers (a 1KB bias DMA queued behind megabytes of x stalls the first
  eviction and with it the whole pipeline).
"""

import numpy as np

import bass_rust
import concourse.bass as bass
import concourse.mybir as mybir
import concourse.tile as tile
from concourse.bass_utils import run_bass_kernel_spmd

B = 2
S = 2048
D = 1024
H = 16
DH = 64
OUT = 1024
NCORES = 8
HPC = H // 4  # heads per core = 4
E = HPC * DH  # 256 ctx dims per core
EV = HPC * (DH + 1)  # 260: v with interleaved ones columns

F16 = mybir.dt.float16
FP32 = mybir.dt.float32

SCALE = 1.0 / float(np.sqrt(DH))

KT = D // 128  # 8 k-tiles for projections
MT = S // 128  # 16 key-token tiles
NQ = S // 512  # 4 query slices of 512
NSTEP = 2 * NQ * MT  # 128 pipeline steps


def _split_waits(nc, maxw=1):
    """This container's walrus rejects instructions carrying more than one
    semaphore wait ("Too many sync wait commands"); hoist extras onto
    standalone same-engine nops, preserving per-engine program order."""
    for bb in nc.main_func.blocks:
        new_il = []
        for inst in bb.instructions:
            si = inst.sync_info
            if si is not None and si.on_wait and len(si.on_wait) > maxw:
                waits = list(si.on_wait)
                for j, w in enumerate(waits[:-maxw]):
                    nop = mybir.InstNoOp(
                        name=f"{inst.name}-ws{j}", ins=[], outs=[], engine=inst.engine
                    )
                    nop.sync_info = bass_rust.SyncInfo(on_wait=[w], on_update=[])
                    new_il.append(nop)
                inst.sync_info = bass_rust.SyncInfo(
                    on_wait=waits[-maxw:], on_update=list(si.on_update)
                )
            new_il.append(inst)
        bb.instructions = new_il


def build_program():
    nc = bass.Bass()

    # All inputs are host-packed into 128-partition-row layouts with long
    # contiguous lines. x additionally interleaves [chunk(4), k(8), 512]
    # along the columns so the first 512-token chunk (everything the first
    # attention steps need) is a small number of whole-row transfers.
    xP = nc.declare_dram_parameter("xP", [128, KT * S], F16, isOutput=False)
    wq = nc.declare_dram_parameter("wq", [128, KT * E], F16, isOutput=False)
    wk = nc.declare_dram_parameter("wk", [128, KT * E], F16, isOutput=False)
    wv = nc.declare_dram_parameter("wv", [128, KT * EV], F16, isOutput=False)
    wo = nc.declare_dram_parameter("wo", [E, OUT], F16, isOutput=False)
    # bq[2] | bk[2] | bo/4[8] merged: one DMA dispatch instead of three
    bqko = nc.declare_dram_parameter("bqko", [128, 12], FP32, isOutput=False)
    bvfp = nc.declare_dram_parameter("bvf", [128, EV], FP32, isOutput=False)
    outT = nc.declare_dram_parameter("outT", [OUT, S], F16, isOutput=True)

    with tile.TileContext(nc) as tc:
        with (
            tc.tile_pool(name="w", bufs=1) as wpool,
            tc.tile_pool(name="work", bufs=3) as work,
            tc.tile_pool(name="cnp", bufs=1) as cnpool,
            tc.tile_pool(name="ps", bufs=2, space="PSUM") as psp,
            tc.tile_pool(name="ctxps", bufs=3, space="PSUM") as ctxp,
            tc.tile_pool(name="pop", bufs=1, space="PSUM") as pop,
        ):
            # ---- persistent SBUF residents ----
            xall = wpool.tile([128, KT * S], F16, tag="xall")

            def xsl(k, t0, width):
                """x slice [128, width] for k-tile k, tokens t0..t0+width
                (must stay inside one 512-token chunk of the packed layout)."""
                c, off = divmod(t0, 512)
                assert off + width <= 512
                base = c * (KT * 512) + k * 512 + off
                return xall[:, base : base + width]

            wqall = wpool.tile([128, KT * E], F16, tag="wqall")
            wkall = wpool.tile([128, KT * E], F16, tag="wkall")
            wvall = wpool.tile([128, KT * EV], F16, tag="wvall")
            wos = [wpool.tile([128, OUT], F16, tag=f"wo{k}", name=f"wo{k}") for k in range(2)]
            wqs = [wqall[:, k * E : (k + 1) * E] for k in range(KT)]
            wks = [wkall[:, k * E : (k + 1) * E] for k in range(KT)]
            wvs = [wvall[:, k * EV : (k + 1) * EV] for k in range(KT)]
            bqko_s = wpool.tile([128, 12], FP32, tag="bqko")
            bvf_s = wpool.tile([128, EV], FP32, tag="bvf")
            bq_s = bqko_s[:, 0:2]
            bk_s = bqko_s[:, 2:4]
            bo_s = bqko_s[:, 4:12]
            qts = [wpool.tile([128, S], F16, tag=f"qt{m}", name=f"qt{m}") for m in range(2)]
            kts = [wpool.tile([128, S], F16, tag=f"kt{m}", name=f"kt{m}") for m in range(2)]
            vts = [wpool.tile([128, EV], F16, tag=f"vt{m}", name=f"vt{m}") for m in range(MT)]
            cns = [cnpool.tile([128, S], F16, tag=f"cn{m}", name=f"cn{m}") for m in range(2)]
            wu = wpool.tile([128, 512], F16, tag="wu")
            ones_f = wpool.tile([1, 64], F16, tag="ones_f")

            # ---- DMA. Each dma_start costs ~640ns of SERIAL dispatch time
            # on its issuing engine's sequencer, so transfers are split
            # across BOTH HWDGE dispatchers (SP + Activation — ACT is idle
            # until the first exp at ~20us). Rings are wire-limited at
            # ~12GB/s each, so every tensor is partition-split into pieces
            # sized to land the critical wave (wq/wk/wv + x chunk 0 —
            # everything q00/k00/v0..3 need, ~2.7MB) across all rings first.
            CW = KT * 512  # 4096 columns per x chunk
            nc.sync.dma_start(out=bqko_s[:], in_=bqko[:])
            # Hoist the ~1.3us ACT table load to program start: walrus
            # inserts it immediately before the FIRST activation in ACT
            # program order, so a dummy exp here (before the scalar-engine
            # DMA dispatches, which stall on ring credits) runs the load at
            # t~5.5 instead of right before exp(0).
            dum = work.tile([1, 8], FP32, tag="dum", bufs=1)
            nc.scalar.activation(
                dum[:], bqko_s[0:1, 0:8], mybir.ActivationFunctionType.Exp
            )
            nc.scalar.dma_start(out=bvf_s[:], in_=bvfp[:])
            # wave 1: x chunk 0 interleaved with wq (q00's inputs land
            # first), then wk, then wv — alternating dispatch engines
            for p in range(8):
                ps_ = slice(p * 16, (p + 1) * 16)
                eng = nc.sync if p % 2 == 0 else nc.scalar
                eng2 = nc.scalar if p % 2 == 0 else nc.sync
                eng.dma_start(out=xall[ps_, 0:CW], in_=xP[ps_, 0:CW])
                eng2.dma_start(out=wqall[ps_, :], in_=wq[ps_, :])
            for p in range(8):
                ps_ = slice(p * 16, (p + 1) * 16)
                eng = nc.sync if p % 2 == 0 else nc.scalar
                eng.dma_start(out=wkall[ps_, :], in_=wk[ps_, :])
            for p in range(4):
                ps_ = slice(p * 32, (p + 1) * 32)
                nc.scalar.dma_start(out=wvall[ps_, :], in_=wv[ps_, :])
            # later waves: x chunks 1-3 + wo (sync only — ACT must be free
            # for the exp stream once it starts)
            for c in range(1, 4):
                for p in range(8):
                    ps_ = slice(p * 16, (p + 1) * 16)
                    nc.sync.dma_start(
                        out=xall[ps_, c * CW : (c + 1) * CW],
                        in_=xP[ps_, c * CW : (c + 1) * CW],
                    )
            for k in range(2):
                for h in range(2):
                    hs_ = slice(h * 64, (h + 1) * 64)
                    nc.sync.dma_start(
                        out=wos[k][hs_, :], in_=wo[k * 128 + h * 64 : k * 128 + (h + 1) * 64, :]
                    )

            nc.vector.memset(wu[:], 0.0)
            nc.vector.memset(ones_f[:], 1.0)
            # Warm the PE clock (HAM un-throttles after ~3.4us of SUSTAINED
            # activity; any ~3.4us idle re-throttles). One long free-running
            # batch reaches warm and carries to the first x pieces, then
            # piece-gated batches keep the PE busy right up to q00.
            wups = psp.tile([128, 1024], FP32, tag="S", name="wups")
            for i in range(18):
                nc.tensor.matmul(
                    wups[:, 0:512], lhsT=wu[:, 0:128], rhs=wu[:], start=True, stop=True
                )
            for pr, nmm in ((slice(0, 32), 6), (slice(32, 64), 6), (slice(64, 128), 8)):
                for i in range(nmm):
                    nc.tensor.matmul(
                        wups[:, 0:512],
                        lhsT=wu[pr, 0:128],
                        rhs=xall[pr, (i % 4) * 512 : (i % 4) * 512 + 512],
                        start=True,
                        stop=True,
                    )

            open_qk = {}

            def qk_half(hp, which, n, half, pool=None, tag=None):
                """Half of a projection accumulation group (4 k-tiles).
                Full 8-MM groups monopolize the PE for ~3.4us and starve the
                exp stream, so fillers emit them as two halves at adjacent
                steps. half=1 closes the group and evicts."""
                w_s, dst, bias = (
                    (wqs, qts, bq_s) if which == "q" else (wks, kts, bk_s)
                )
                key = (hp, which, n)
                if half == 0:
                    p = pool or pop
                    ps = p.tile(
                        [128, 512], FP32, tag=tag or "po", name=f"ps_{which}{hp}{n}"
                    )
                    open_qk[key] = ps
                else:
                    ps = open_qk.pop(key)
                for k in range(half * 4, half * 4 + 4):
                    nc.tensor.matmul(
                        ps[:],
                        lhsT=w_s[k][:, hp * 128 : (hp + 1) * 128],
                        rhs=xsl(k, n * 512, 512),
                        start=(k == 0),
                        stop=(k == KT - 1),
                    )
                if half == 1:
                    # tensor_tensor PSUM->fp16 is the fast DVE path; the
                    # bias column broadcasts along the free axis.
                    nc.vector.tensor_add(
                        dst[hp][:, n * 512 : (n + 1) * 512],
                        ps[:],
                        bias[:, hp : hp + 1].to_broadcast((128, 512)),
                    )

            def qk_group(hp, which, n, pool=None, tag=None):
                qk_half(hp, which, n, 0, pool=pool, tag=tag)
                qk_half(hp, which, n, 1)

            def v_group(m):
                """v_ext rows m*128..m*128+127 (token-major). The ones
                columns (and bv) come from the bvf addend at eviction."""
                ps = pop.tile([128, 512], FP32, tag="po", name=f"psv{m}")
                for k in range(KT):
                    nc.tensor.matmul(
                        ps[:, :EV],
                        lhsT=xsl(k, m * 128, 128),
                        rhs=wvs[k][:],
                        start=(k == 0),
                        stop=(k == KT - 1),
                    )
                nc.vector.tensor_add(vts[m][:], ps[:, :EV], bvf_s[:])

            def scores_exp(hp, nq, m):
                """Scores pair for one m-tile + the exp on ScalarE."""
                sps = psp.tile([128, 1024], FP32, tag="S", name="sps")
                nc.tensor.matmul(
                    sps[:, 0:512],
                    lhsT=kts[hp][0:64, m * 128 : (m + 1) * 128],
                    rhs=qts[hp][0:64, nq * 512 : (nq + 1) * 512],
                    start=True,
                    stop=True,
                )
                nc.tensor.matmul(
                    sps[:, 512:1024],
                    lhsT=kts[hp][64:128, m * 128 : (m + 1) * 128],
                    rhs=qts[hp][64:128, nq * 512 : (nq + 1) * 512],
                    start=True,
                    stop=True,
                )
                ee = work.tile([128, 1024], F16, tag="E")
                nc.scalar.activation(
                    ee[:], sps[:], mybir.ActivationFunctionType.Exp, scale=SCALE
                )
                return ee

            def ctx_mms(ctx_pair, hp, m, ee):
                ctx_a, ctx_b = ctx_pair
                ha = 2 * hp
                nc.tensor.matmul(
                    ctx_a[:],
                    lhsT=vts[m][:, ha * 65 : ha * 65 + 65],
                    rhs=ee[:, 0:512],
                    start=(m == 0),
                    stop=(m == MT - 1),
                )
                nc.tensor.matmul(
                    ctx_b[:],
                    lhsT=vts[m][:, (ha + 1) * 65 : (ha + 1) * 65 + 65],
                    rhs=ee[:, 512:1024],
                    start=(m == 0),
                    stop=(m == MT - 1),
                )

            def norm_p1(ctx_pair):
                """Evict the ctx pair to SBUF with fast fp32 copies (frees
                the PSUM banks for the next slice's accumulators)."""
                ctx_a, ctx_b = ctx_pair
                cs = work.tile([65, 1024], FP32, tag="cs", bufs=3, name="cs_ab")
                nc.vector.tensor_copy(cs[:, 0:512], ctx_a[:])
                nc.vector.tensor_copy(cs[:, 512:1024], ctx_b[:])
                return cs

            def norm_recip(cs):
                """1/s = exp(-ln(s)) in two batched ACT ops (same table set
                as the scores exp; the microcoded DVE reciprocal measures
                3.2us per [1,512] and is undercosted by the Tile scheduler,
                stalling the PE queue). Emitted two steps after the slice
                boundary so the boundary exp stream isn't perturbed."""
                ln = work.tile([1, 1024], FP32, tag="lns", bufs=2, name="ln_ab")
                nc.scalar.activation(ln[:], cs[64:65, :], mybir.ActivationFunctionType.Ln)
                r16 = work.tile([1, 1024], F16, tag="r16", bufs=2, name="r16")
                nc.scalar.activation(
                    r16[:], ln[:], mybir.ActivationFunctionType.Exp, scale=-1.0
                )
                return r16

            def norm_p2(nrm, hp, nq, a):
                """Deferred: broadcast one head's 1/rowsum across partitions
                (K=1 fp16 matmul into a pop-pool PSUM tile) and scale."""
                cs, r16 = nrm
                bc = pop.tile([128, 512], FP32, tag="po", name=f"bc{hp}{nq}{a}")
                nc.tensor.matmul(
                    bc[0:64, :],
                    lhsT=ones_f[:],
                    rhs=r16[0:1, 512 * a : 512 * a + 512],
                    start=True,
                    stop=True,
                )
                nc.vector.tensor_mul(
                    cns[hp][64 * a : 64 * a + 64, nq * 512 : (nq + 1) * 512],
                    cs[0:64, 512 * a : 512 * a + 512],
                    bc[0:64, :],
                )

            def out_proj_group(n, mo, pool=None, tag=None):
                """One [128,512] tile of the partial out^T for query slice n."""
                p = pool or pop
                ps = p.tile([128, 512], FP32, tag=tag or "po", name=f"ps_o{n}_{mo}")
                for k in range(2):
                    nc.tensor.matmul(
                        ps[:],
                        lhsT=wos[k][:, mo * 128 : (mo + 1) * 128],
                        rhs=cns[k][:, n * 512 : (n + 1) * 512],
                        start=(k == 0),
                        stop=(k == 1),
                    )
                ot = work.tile([128, 512], F16, tag="ot")
                nc.vector.tensor_add(
                    ot[:], ps[:], bo_s[:, mo : mo + 1].to_broadcast((128, 512))
                )
                nc.sync.dma_start(
                    out=outT[mo * 128 : (mo + 1) * 128, n * 512 : (n + 1) * 512],
                    in_=ot[:],
                )

            # ---- filler schedule (step -> thunks), with deadlines:
            # v(m) before ctx(m) at step m+1; k0n before scores(4n);
            # q0n before slice n (step 16n); hp=1 projections before step 64+;
            # p2(X) after slice X's recip; out(n) after p2(slice 4+n).
            normed = {}
            fills = {}

            def at(s, th):
                fills.setdefault(s, []).append(th)

            for m in range(1, MT):
                at(m - 1, lambda m=m: v_group(m))

            def qk_at(s, hp, which, n):
                at(s, lambda: qk_half(hp, which, n, 0))
                at(s + 1, lambda: qk_half(hp, which, n, 1))

            qk_at(1, 0, "k", 1)
            qk_at(5, 0, "k", 2)
            qk_at(8, 0, "k", 3)
            qk_at(11, 0, "q", 1)
            qk_at(18, 0, "q", 2)
            qk_at(22, 1, "k", 0)
            qk_at(26, 1, "k", 1)
            qk_at(34, 0, "q", 3)
            qk_at(38, 1, "k", 2)
            qk_at(42, 1, "k", 3)
            qk_at(50, 1, "q", 0)
            qk_at(54, 1, "q", 1)
            qk_at(66, 1, "q", 2)
            qk_at(82, 1, "q", 3)
            slices = [(0, 0), (0, 1), (0, 2), (0, 3), (1, 0), (1, 1), (1, 2), (1, 3)]

            def recip_fill(X):
                normed[X][1] = norm_recip(normed[X][0])

            for X in range(7):
                hpX, nqX = slices[X]
                at(16 * (X + 1) + 2, lambda X=X: recip_fill(X))
                at(16 * (X + 1) + 4, lambda X=X, h=hpX, n=nqX: norm_p2(normed[X], h, n, 0))
                at(16 * (X + 1) + 6, lambda X=X, h=hpX, n=nqX: norm_p2(normed[X], h, n, 1))
            # out groups: six spread mid-slice, the last two placed right at
            # the NEXT slice boundary to cover the ln/exp ACT insertion
            for n, base, bnd in [(0, 88, 96), (1, 104, 112), (2, 119, None)]:
                for mo in range(6):
                    at(base + mo, lambda n=n, mo=mo: out_proj_group(n, mo))
                if bnd is not None:
                    at(bnd + 1, lambda n=n: out_proj_group(n, 6))
                    at(bnd + 2, lambda n=n: out_proj_group(n, 7))
                else:
                    at(base + 6, lambda n=n: out_proj_group(n, 6))
                    at(base + 7, lambda n=n: out_proj_group(n, 7))

            # ---- prologue compute ----
            qk_group(0, "q", 0, pool=psp, tag="S")
            qk_group(0, "k", 0, pool=psp, tag="S")
            v_group(0)

            # ---- the pipelined stream ----
            ctx_pair = None
            prev = None  # (hp, m, ee)
            for s in range(NSTEP):
                sl, m = divmod(s, MT)
                hp, nq = slices[sl]
                for f in fills.get(s, ()):
                    f()
                ee = scores_exp(hp, nq, m)
                if prev is not None:
                    phn, pm, pee = prev
                    if pm == 0:
                        ctx_pair = (
                            ctxp.tile([65, 512], FP32, tag="ctx", name="ctx_a"),
                            ctxp.tile([65, 512], FP32, tag="ctx", name="ctx_b"),
                        )
                    ctx_mms(ctx_pair, phn, pm, pee)
                    if pm == MT - 1:
                        normed[(s - 1) // MT] = [norm_p1(ctx_pair), None]
                prev = (hp, m, ee)
            # final ctx + normalization + out projection
            phn, pm, pee = prev
            ctx_mms(ctx_pair, phn, pm, pee)
            cs7 = norm_p1(ctx_pair)
            # keep the PE warm through the final eviction + ln/exp chain
            wups2 = psp.tile([128, 1024], FP32, tag="S", name="wups2")
            for i in range(4):
                nc.tensor.matmul(
                    wups2[:, 0:512], lhsT=wu[:, 0:128], rhs=wu[:], start=True, stop=True
                )
            r16_7 = norm_recip(cs7)
            wups3 = psp.tile([128, 1024], FP32, tag="S", name="wups3")
            for i in range(10):
                nc.tensor.matmul(
                    wups3[:, 0:512], lhsT=wu[:, 0:128], rhs=wu[:], start=True, stop=True
                )
            norm_p2((cs7, r16_7), 1, 3, 0)
            norm_p2((cs7, r16_7), 1, 3, 1)
            # distribute the final groups over the (now free) scores pool's
            # two buffers so they pipeline instead of serializing on pop
            for mo in range(OUT // 128):
                if mo in (0, 4):
                    out_proj_group(3, mo)
                else:
                    out_proj_group(3, mo, pool=psp, tag="S")

    _split_waits(nc)
    return nc


_PROGRAM = None


def _get_program():
    global _PROGRAM
    if _PROGRAM is None:
        _PROGRAM = build_program()
    return _PROGRAM


def _shard_inputs(x, Wq, bq, Wk, bk, Wv, bv, Wo, bo):
    f16 = np.float16
    in_maps = []
    for c in range(NCORES):
        b = c // 4
        g = c % 4
        hs = slice(g * HPC, (g + 1) * HPC)

        # x packed [128, chunk(4) x k(8) x 512]: row p, col c*4096+k*512+t'
        # holds x[b][c*512+t', k*128+p]
        xTc = np.ascontiguousarray(
            x[b].T.reshape(KT, 128, NQ, 512)
            .transpose(1, 2, 0, 3)
            .reshape(128, KT * S)
        ).astype(f16)
        # packed weight layouts: row p, col-block k holds W[k*128+p, :]
        wq_c = np.ascontiguousarray(
            Wq[hs].transpose(1, 0, 2).reshape(KT, 128, E).transpose(1, 0, 2).reshape(128, KT * E)
        ).astype(f16)
        wk_c = np.ascontiguousarray(
            Wk[hs].transpose(1, 0, 2).reshape(KT, 128, E).transpose(1, 0, 2).reshape(128, KT * E)
        ).astype(f16)
        wv_d = np.zeros((D, EV), dtype=np.float32)
        bvf_row = np.zeros((EV,), dtype=np.float32)
        for h in range(HPC):
            wv_d[:, h * 65 : h * 65 + 64] = Wv[hs][h]
            bvf_row[h * 65 : h * 65 + 64] = bv[hs][h]
            bvf_row[h * 65 + 64] = 1.0
        wv_c = np.ascontiguousarray(
            wv_d.reshape(KT, 128, EV).transpose(1, 0, 2).reshape(128, KT * EV)
        )
        bvf_c = np.ascontiguousarray(np.tile(bvf_row[None, :], (128, 1))).astype(
            np.float32
        )
        wo_c = np.ascontiguousarray(Wo[g * E : (g + 1) * E, :]).astype(f16)
        bq_c = bq[hs].reshape(E // 128, 128).T.astype(np.float32)
        bk_c = bk[hs].reshape(E // 128, 128).T.astype(np.float32)
        bo_c = (bo.astype(np.float32) * 0.25).reshape(OUT // 128, 128).T
        bqko_c = np.ascontiguousarray(
            np.concatenate([bq_c, bk_c, bo_c], axis=1)
        ).astype(np.float32)

        in_maps.append(
            {
                "xP": xTc,
                "wq": wq_c,
                "wk": wk_c,
                "wv": wv_c.astype(f16),
                "wo": wo_c,
                "bqko": bqko_c,
                "bvf": bvf_c,
            }
        )
    return in_maps


def kernel(x, Wq, bq, Wk, bk, Wv, bv, Wo, bo, _trace=False, _result_box=None):
    in_maps = _shard_inputs(
        np.asarray(x, np.float32),
        np.asarray(Wq, np.float32),
        np.asarray(bq, np.float32),
        np.asarray(Wk, np.float32),
        np.asarray(bk, np.float32),
        np.asarray(Wv, np.float32),
        np.asarray(bv, np.float32),
        np.asarray(Wo, np.float32),
        np.asarray(bo, np.float32),
    )
    nc = _get_program()
    res = run_bass_kernel_spmd(nc, in_maps, list(range(NCORES)), trace=_trace)
    if _result_box is not None:
        _result_box.append(res)

    out = np.empty((B, S, OUT), dtype=np.float32)
    for b in range(B):
        acc = res.results[4 * b]["outT"].astype(np.float32)
        for g in range(1, 4):
            acc += res.results[4 * b + g]["outT"].astype(np.float32)
        out[b] = acc.T
    return out
